# revision 30
# baseline (speedup 1.0000x reference)
"""Trainium2 Bass kernel for nn_Block_11897059410591 (MLA transformer block).

Sharding over 8 NeuronCores: core c = (batch b=c//2, head-half h0=(c%2)*8).
Each core computes LN1/kvd/kvu/RoPE for its whole batch, causal attention for
its 8 heads, a partial output projection (contracted over its heads) that is
pair-AllReduced, then the FFN with d_ff split in half across the pair and a
second pair-AllReduce. Both cores of a pair end with the identical full-batch
output; the host keeps the even core's copy. The back half (proj -> residual
-> FFN -> output) is pipelined over 4 token slabs so the AllReduces overlap
with compute.
"""
import sys

if "/opt/trn_rl_repo" not in sys.path:
    sys.path.insert(0, "/opt/trn_rl_repo")

import numpy as np
import ml_dtypes


def _ensure_ntff_hook():
    """antenv.axon_hooks is missing in this image; shim it so
    run_bass_kernel_spmd(trace=True) can capture NTFF profiles."""
    try:
        from antenv import axon_hooks  # noqa: F401
        return
    except ImportError:
        pass
    try:
        import types
        import importlib.util
        m = types.ModuleType("antenv.axon_hooks")
        _hook = [None]
        m.set_axon_ntff_profile_hook = lambda h: _hook.__setitem__(0, h)
        m.get_axon_ntff_profile_hook = lambda: _hook[0]
        sys.modules["antenv.axon_hooks"] = m
        import antenv
        antenv.axon_hooks = m
        spec = importlib.util.spec_from_file_location(
            "_trn_boot_shim", "/root/.axon_site/trn_agent_boot/trn_boot.py")
        tb = importlib.util.module_from_spec(spec)
        spec.loader.exec_module(tb)
        hook = tb._ntff_profile_via_ctypes("/opt/axon/libaxon_pjrt.so")
        m.set_axon_ntff_profile_hook(hook)
    except Exception as e:  # degrade to trace-less operation
        print(f"ntff hook shim failed ({e}); tracing disabled", file=sys.stderr)


_ensure_ntff_hook()

import concourse.bass as bass
import concourse.mybir as mybir
import concourse.tile as tile
from concourse import bacc
from concourse.bass_utils import run_bass_kernel_spmd
from concourse.masks import make_identity

F32 = mybir.dt.float32
BF = mybir.dt.bfloat16
BF16 = ml_dtypes.bfloat16
AF = mybir.ActivationFunctionType
ALU = mybir.AluOpType

B, T, C = 4, 2048, 1024
H, D, R, FF = 16, 64, 512, 4096
HL = 8              # heads per core
HD = HL * D         # 512
FH = FF // 2        # 2048, d_ff half per core
P = 128
NT = T // P         # 16 token chunks
NCC = C // P        # 8 C chunks
NRC = R // P        # 4 R chunks
NSL = 4             # token slabs for the back half
SLB = T // NSL      # 512 tokens per slab
LN_EPS = 1e-5

TRACE = False
_CACHE = {}


def _rope_tables():
    inv_freq = 1.0 / (10000.0 ** (np.arange(0, D, 2, dtype=np.float32) / D))
    t = np.arange(T, dtype=np.float32)
    freqs = np.outer(t, inv_freq)
    emb = np.concatenate([freqs, freqs], axis=-1)  # [T, D]
    cos = np.cos(emb).astype(np.float32)
    sin = np.sin(emb).astype(np.float32)
    sinf = sin.copy()
    sinf[:, : D // 2] = -sinf[:, : D // 2]
    return cos, sinf


def _build(flags):
    (ln1_triv, kvln_triv, ln2_triv, pb0, f1b0, f2b0) = flags
    nc = bacc.Bacc("TRN2", target_bir_lowering=False, debug=False,
                   enable_asserts=False, num_devices=8)

    x_d = nc.dram_tensor("x_loc", [T, C], F32, kind="ExternalInput").ap()
    qw_d = nc.dram_tensor("qw_loc", [C, HD], BF, kind="ExternalInput").ap()
    kvd_d = nc.dram_tensor("kvd_w", [C, R], BF, kind="ExternalInput").ap()
    kvuk_d = nc.dram_tensor("kvu_k", [R, HD], BF, kind="ExternalInput").ap()
    kvuv_d = nc.dram_tensor("kvu_v", [R, HD], BF, kind="ExternalInput").ap()
    pw_d = nc.dram_tensor("proj_w_loc", [HD, C], BF, kind="ExternalInput").ap()
    f1_d = nc.dram_tensor("f1_w_loc", [C, FH], BF, kind="ExternalInput").ap()
    f2_d = nc.dram_tensor("f2_w_loc", [FH, C], BF, kind="ExternalInput").ap()
    cos_d = nc.dram_tensor("cos_t", [T, D], F32, kind="ExternalInput").ap()
    sinf_d = nc.dram_tensor("sinf_t", [T, D], F32, kind="ExternalInput").ap()
    out_d = nc.dram_tensor("out_loc", [T, C], F32, kind="ExternalOutput").ap()

    opt_ins = {}
    if not ln1_triv:
        opt_ins["ln1_w"] = nc.dram_tensor("ln1_w", [C], F32, kind="ExternalInput").ap()
        opt_ins["ln1_b"] = nc.dram_tensor("ln1_b", [C], F32, kind="ExternalInput").ap()
    if not kvln_triv:
        opt_ins["kvln_w"] = nc.dram_tensor("kvln_w", [R], F32, kind="ExternalInput").ap()
        opt_ins["kvln_b"] = nc.dram_tensor("kvln_b", [R], F32, kind="ExternalInput").ap()
    if not ln2_triv:
        opt_ins["ln2_w"] = nc.dram_tensor("ln2_w", [C], F32, kind="ExternalInput").ap()
        opt_ins["ln2_b"] = nc.dram_tensor("ln2_b", [C], F32, kind="ExternalInput").ap()
    if not pb0:
        opt_ins["proj_b"] = nc.dram_tensor("proj_b", [C], F32, kind="ExternalInput").ap()
    if not f1b0:
        opt_ins["f1_b"] = nc.dram_tensor("f1_b_loc", [FH], F32, kind="ExternalInput").ap()
    if not f2b0:
        opt_ins["f2_b"] = nc.dram_tensor("f2_b", [C], F32, kind="ExternalInput").ap()

    # internal DRAM
    cc1_in = [nc.dram_tensor(f"cc1_in{n}", [C, SLB], BF).ap()
              for n in range(NSL)]
    cc1_out = [nc.dram_tensor(f"cc1_out{n}", [C, SLB], BF).ap()
               for n in range(NSL)]
    cc2_in = [nc.dram_tensor(f"cc2_in{n}", [C, SLB], BF).ap()
              for n in range(NSL)]
    cc2_out = [nc.dram_tensor(f"cc2_out{n}", [C, SLB], BF).ap()
               for n in range(NSL)]
    x2_dram = [nc.dram_tensor(f"x2_bounce{n}", [SLB, C], F32).ap()
               for n in range(NSL)]
    groups = [[0, 1], [2, 3], [4, 5], [6, 7]]

    def bcast_free(ap2d, n, width):
        """[P, width] AP -> [P, n, width] with 0-step middle dim."""
        return bass.AP(tensor=ap2d.tensor, offset=ap2d.offset,
                       ap=[ap2d.ap[0], [0, n], [1, width]])

    def ln_stats(pool, src_ap, width, eps_t):
        """Per-partition (mean, rstd) of src_ap [P, width]."""
        ngr = (width + 511) // 512
        st6 = pool.tile([P, ngr, 6], F32, tag="st6")
        sv = src_ap.rearrange("p (g d) -> p g d", g=ngr)
        for g in range(ngr):
            nc.vector.bn_stats(out=st6[:, g, :], in_=sv[:, g, :])
        mv = pool.tile([P, 2], F32, tag="mv")
        nc.vector.bn_aggr(out=mv, in_=st6)
        nc.scalar.activation(out=mv[:, 1:2], in_=mv[:, 1:2], func=AF.Sqrt,
                             bias=eps_t, scale=1.0)
        nc.vector.reciprocal(out=mv[:, 1:2], in_=mv[:, 1:2])
        return mv

    from contextlib import ExitStack
    with tile.TileContext(nc) as tc:
        with ExitStack() as ctx:
            const = ctx.enter_context(tc.tile_pool(name="const", bufs=1))
            ident = const.tile([P, P], BF)
            make_identity(nc, ident)
            eps_t = const.tile([P, 1], F32)
            nc.vector.memset(eps_t, LN_EPS)
            # S^T diagonal causal mask: keep (0) where col(q) >= row(k)
            maskT = const.tile([P, P], F32)
            nc.gpsimd.memset(maskT, 0.0)
            nc.gpsimd.affine_select(out=maskT, in_=maskT, compare_op=ALU.is_ge,
                                    fill=-1e9, base=0, pattern=[[1, P]],
                                    channel_multiplier=-1)
            mask_full = const.tile([P, P], F32)
            nc.vector.memset(mask_full, -1e9)

            def dram_row_bcast(name, ap1d, width):
                t = const.tile([P, width], F32, name=name)
                src = bass.AP(tensor=ap1d.tensor, offset=ap1d.offset,
                              ap=[[0, P], [1, width]])
                nc.sync.dma_start(out=t, in_=src)
                return t

            ln1_wt = ln1_bt = ln2_wt = ln2_bt = kvln_wt = kvln_bt = None
            if not ln1_triv:
                ln1_wt = dram_row_bcast("ln1w_b", opt_ins["ln1_w"], C)
                ln1_bt = dram_row_bcast("ln1b_b", opt_ins["ln1_b"], C)
            if not kvln_triv:
                kvln_wt = dram_row_bcast("kvlnw_b", opt_ins["kvln_w"], R)
                kvln_bt = dram_row_bcast("kvlnb_b", opt_ins["kvln_b"], R)
            if not ln2_triv:
                ln2_wt = dram_row_bcast("ln2w_b", opt_ins["ln2_w"], C)
                ln2_bt = dram_row_bcast("ln2b_b", opt_ins["ln2_b"], C)
            projb_t = f1b_t = f2b_t = None
            if not pb0:
                projb_t = const.tile([P, NCC], F32, name="projb")
                nc.sync.dma_start(out=projb_t, in_=opt_ins["proj_b"].rearrange(
                    "(m p) -> p m", p=P))
            if not f1b0:
                f1b_t = const.tile([P, FH // P], F32, name="f1b")
                nc.sync.dma_start(out=f1b_t, in_=opt_ins["f1_b"].rearrange(
                    "(m p) -> p m", p=P))
            if not f2b0:
                f2b_t = const.tile([P, NCC], F32, name="f2b")
                nc.sync.dma_start(out=f2b_t, in_=opt_ins["f2_b"].rearrange(
                    "(m p) -> p m", p=P))

            def load_chunks(pool, dram_ap, nk, width, name):
                t = pool.tile([P, nk, width], BF, name=name)
                for k in range(nk):
                    nc.sync.dma_start(out=t[:, k, :],
                                      in_=dram_ap[k * P:(k + 1) * P, :])
                return t

            # ---- long-lived pools (created early; closed last, LIFO) ------
            pool_xs = ctx.enter_context(tc.tile_pool(name="xs2", bufs=2))
            pool_h2T = ctx.enter_context(tc.tile_pool(name="h2T", bufs=4))
            ps_tr1 = ctx.enter_context(
                tc.tile_pool(name="pstr1", bufs=2, space="PSUM"))
            h2T_sl = [None] * NSL
            fivb_pools = {}

            def emit_5b(n):
                """Residual + LN2 + h2T for token slab n."""
                h2T = pool_h2T.tile([P, NCC, SLB], BF, tag="h2T")
                h2T_sl[n] = h2T
                saT = fivb_pools["saT"].tile([P, NCC, SLB], BF, tag="saT",
                                             name="saT")
                for kc in range(NCC):
                    nc.sync.dma_start(
                        out=saT[:, kc, :],
                        in_=cc1_out[n][kc * P:(kc + 1) * P, :])
                for itl in range(SLB // P):
                    it = n * (SLB // P) + itl
                    sl = slice(it * P, (it + 1) * P)
                    lsl = slice(itl * P, (itl + 1) * P)
                    pt1 = ps_tr1.tile([P, C], BF, tag="p1k")
                    for kc in range(NCC):
                        nc.tensor.transpose(pt1[:, kc * P:(kc + 1) * P],
                                            saT[:, kc, lsl], ident)
                    xt = pool_xs.tile([P, C], F32, tag="xt2")
                    nc.sync.dma_start(out=xt, in_=x_d[sl, :])
                    x2t = fivb_pools["x2"].tile([P, C], F32, tag="x2t",
                                                name="x2t")
                    nc.vector.tensor_add(x2t, xt, pt1)
                    nc.sync.dma_start(out=x2_dram[n][lsl, :], in_=x2t)
                    mv = ln_stats(fivb_pools["st2"], x2t, C, eps_t)
                    h2 = fivb_pools["tok2"].tile([P, C], BF, tag="h2", name="h2")
                    nc.vector.tensor_scalar(out=h2, in0=x2t,
                                            scalar1=mv[:, 0:1],
                                            scalar2=mv[:, 1:2],
                                            op0=ALU.subtract, op1=ALU.mult)
                    if ln2_wt is not None:
                        nc.vector.tensor_mul(h2, h2, ln2_wt)
                        nc.vector.tensor_add(h2, h2, ln2_bt)
                    pt2 = ps_tr1.tile([P, C], BF, tag="p1k")
                    for kc in range(NCC):
                        nc.tensor.transpose(pt2[:, kc * P:(kc + 1) * P],
                                            h2[:, kc * P:(kc + 1) * P],
                                            ident)
                    nc.scalar.copy(
                        out=h2T[:, :, lsl],
                        in_=pt2.rearrange("p (kc t) -> p kc t", kc=NCC))

            # ---------------- qkv/att scope --------------------------------
            mid = ExitStack()
            pool_qkv = mid.enter_context(tc.tile_pool(name="qkv", bufs=1))
            pool_att = mid.enter_context(tc.tile_pool(name="att", bufs=2))
            qT = pool_qkv.tile([P, HL // 2, T], BF)   # [(2h,64d), hp, T]
            kT = pool_qkv.tile([P, HL // 2, T], BF)
            vaug = pool_qkv.tile([P, NT, HL, D + 1], BF)

            # ------- fused prep: LN1 / h^T / ckv^T / q / k / v per chunk ---
            with ExitStack() as prep:
                pool_xsp = prep.enter_context(tc.tile_pool(name="xs", bufs=3))
                pool_stp = prep.enter_context(tc.tile_pool(name="st", bufs=4))
                pool_tokp = prep.enter_context(tc.tile_pool(name="tok", bufs=3))
                pool_w = prep.enter_context(tc.tile_pool(name="wts", bufs=1))
                pool_cs = prep.enter_context(tc.tile_pool(name="cs", bufs=3))
                pool_ro = prep.enter_context(tc.tile_pool(name="ro", bufs=3))
                pool_hts = prep.enter_context(tc.tile_pool(name="hts", bufs=4))
                pool_cks = prep.enter_context(tc.tile_pool(name="cks", bufs=4))
                ps_big = prep.enter_context(
                    tc.tile_pool(name="psbig", bufs=3, space="PSUM"))
                ps_tr = prep.enter_context(
                    tc.tile_pool(name="pstr", bufs=3, space="PSUM"))

                qw_sb = load_chunks(pool_w, qw_d, NCC, HD, "qw")
                kvdw_sb = load_chunks(pool_w, kvd_d, NCC, R, "kvdw")
                kvuk_sb = load_chunks(pool_w, kvuk_d, NRC, HD, "kvuk")
                kvuv_sb = load_chunks(pool_w, kvuv_d, NRC, HD, "kvuv")

                for it in range(NT):
                    sl = slice(it * P, (it + 1) * P)
                    xt = pool_xsp.tile([P, C], F32)
                    nc.sync.dma_start(out=xt, in_=x_d[sl, :])
                    mv = ln_stats(pool_stp, xt, C, eps_t)
                    ht = pool_tokp.tile([P, C], BF, tag="ht")
                    nc.vector.tensor_scalar(out=ht, in0=xt,
                                            scalar1=mv[:, 0:1], scalar2=mv[:, 1:2],
                                            op0=ALU.subtract, op1=ALU.mult)
                    if ln1_wt is not None:
                        nc.vector.tensor_mul(ht, ht, ln1_wt)
                        nc.vector.tensor_add(ht, ht, ln1_bt)
                    hTt = pool_hts.tile([P, NCC, P], BF, tag="hTt")
                    for kc in range(NCC):
                        pt = ps_tr.tile([P, P], BF, tag="ptr")
                        nc.tensor.transpose(pt, ht[:, kc * P:(kc + 1) * P], ident)
                        if kc % 2 == 0:
                            nc.scalar.copy(out=hTt[:, kc, :], in_=pt)
                        else:
                            nc.vector.tensor_copy(out=hTt[:, kc, :], in_=pt)

                    # ckv = LN(h @ kvd_w), transposed
                    ps = ps_big.tile([P, R], F32, tag="psb")
                    for kc in range(NCC):
                        nc.tensor.matmul(ps, lhsT=hTt[:, kc, :],
                                         rhs=kvdw_sb[:, kc, :],
                                         start=(kc == 0), stop=(kc == NCC - 1))
                    mv = ln_stats(pool_stp, ps, R, eps_t)
                    ct = pool_tokp.tile([P, R], BF, tag="ckvtok")
                    nc.vector.tensor_scalar(out=ct, in0=ps,
                                            scalar1=mv[:, 0:1], scalar2=mv[:, 1:2],
                                            op0=ALU.subtract, op1=ALU.mult)
                    if kvln_wt is not None:
                        nc.vector.tensor_mul(ct, ct, kvln_wt)
                        nc.vector.tensor_add(ct, ct, kvln_bt)
                    ckvTt = pool_cks.tile([P, NRC, P], BF, tag="ckvTt")
                    for rc in range(NRC):
                        pt = ps_tr.tile([P, P], BF, tag="ptr")
                        nc.tensor.transpose(pt, ct[:, rc * P:(rc + 1) * P], ident)
                        if rc % 2 == 0:
                            nc.scalar.copy(out=ckvTt[:, rc, :], in_=pt)
                        else:
                            nc.vector.tensor_copy(out=ckvTt[:, rc, :], in_=pt)

                    cos_sb = pool_cs.tile([P, D], F32, tag="cos")
                    nc.sync.dma_start(out=cos_sb, in_=cos_d[sl, :])
                    sinf_sb = pool_cs.tile([P, D], F32, tag="sinf")
                    nc.sync.dma_start(out=sinf_sb, in_=sinf_d[sl, :])

                    for which in ("q", "k"):
                        ps = ps_big.tile([P, HD], F32, tag="psb")
                        if which == "q":
                            for kc in range(NCC):
                                nc.tensor.matmul(ps, lhsT=hTt[:, kc, :],
                                                 rhs=qw_sb[:, kc, :],
                                                 start=(kc == 0),
                                                 stop=(kc == NCC - 1))
                        else:
                            for rc in range(NRC):
                                nc.tensor.matmul(ps, lhsT=ckvTt[:, rc, :],
                                                 rhs=kvuk_sb[:, rc, :],
                                                 start=(rc == 0),
                                                 stop=(rc == NRC - 1))
                        psv = ps.rearrange("p (h d) -> p h d", d=D)
                        t1 = pool_ro.tile([P, HL, D], BF, tag="t1")
                        nc.vector.tensor_mul(t1, psv, bcast_free(cos_sb, HL, D))
                        t2 = pool_ro.tile([P, HL, D], BF, tag="t2")
                        half = D // 2
                        sfv = sinf_sb
                        nc.vector.tensor_mul(
                            t2[:, :, 0:half],
                            bass.AP(tensor=psv.tensor, offset=psv.offset + half,
                                    ap=[psv.ap[0], [D, HL], [1, half]]),
                            bass.AP(tensor=sfv.tensor, offset=sfv.offset,
                                    ap=[sfv.ap[0], [0, HL], [1, half]]))
                        nc.vector.tensor_mul(
                            t2[:, :, half:D],
                            bass.AP(tensor=psv.tensor, offset=psv.offset,
                                    ap=[psv.ap[0], [D, HL], [1, half]]),
                            bass.AP(tensor=sfv.tensor, offset=sfv.offset + half,
                                    ap=[sfv.ap[0], [0, HL], [1, half]]))
                        ro = pool_ro.tile([P, HL, D], BF, tag="ro")
                        nc.vector.tensor_add(ro, t1, t2)
                        dstT = qT if which == "q" else kT
                        for h in range(HL):
                            hp, hr = h // 2, (h % 2) * 64
                            pt = ps_tr.tile([64, P], BF, tag="ptr")
                            nc.tensor.transpose(pt, ro[:, h, :], ident)
                            if h % 2 == 0:
                                nc.scalar.copy(out=dstT[0:64, hp, sl], in_=pt)
                            else:
                                nc.vector.tensor_copy(out=dstT[64:128, hp, sl],
                                                      in_=pt)

                    # v (no rope) -> token-major vaug
                    ps = ps_big.tile([P, HD], F32, tag="psb")
                    for rc in range(NRC):
                        nc.tensor.matmul(ps, lhsT=ckvTt[:, rc, :],
                                         rhs=kvuv_sb[:, rc, :],
                                         start=(rc == 0), stop=(rc == NRC - 1))
                    nc.vector.memset(vaug[:, it, :, D:D + 1], 1.0)
                    nc.scalar.copy(out=vaug[:, it, :, 0:D],
                                   in_=ps.rearrange("p (h d) -> p h d", d=D))

            # ---- Phase 3+5a+5b: attention / proj / residual interleaved ---
            with ExitStack() as attn:
                pool_p = attn.enter_context(tc.tile_pool(name="pexp", bufs=3))
                pool_s = attn.enter_context(tc.tile_pool(name="srow", bufs=4))
                pool_bc = attn.enter_context(tc.tile_pool(name="bc", bufs=4))
                pool_pw = attn.enter_context(tc.tile_pool(name="pw", bufs=1))
                pool_sa = attn.enter_context(tc.tile_pool(name="sa", bufs=3))
                ps_s = attn.enter_context(
                    tc.tile_pool(name="pss", bufs=2, space="PSUM"))
                ps_o = attn.enter_context(
                    tc.tile_pool(name="pso", bufs=2, space="PSUM"))
                fivb_pools["saT"] = attn.enter_context(
                    tc.tile_pool(name="saT", bufs=2))
                fivb_pools["x2"] = attn.enter_context(
                    tc.tile_pool(name="x2", bufs=2))
                fivb_pools["st2"] = attn.enter_context(
                    tc.tile_pool(name="st2", bufs=4))
                fivb_pools["tok2"] = attn.enter_context(
                    tc.tile_pool(name="tok2", bufs=2))
                pw_sb = load_chunks(pool_pw, pw_d, NRC, C, "pw")
                NPAIR = NT // 2
                att_cur = [None]  # per-slab [(2h,64d), hp, SLB] tile

                def proj_slab(n):
                    att = att_cur[0]
                    for mt in range(NCC):
                        msl = slice(mt * P, (mt + 1) * P)
                        ps = ps_s.tile([P, SLB], F32, tag="pss")
                        for kc in range(NRC):
                            nc.tensor.matmul(ps, lhsT=pw_sb[:, kc, msl],
                                             rhs=att[:, kc, :],
                                             start=(kc == 0),
                                             stop=(kc == NRC - 1))
                        sa_t = pool_sa.tile([P, SLB], BF, tag="sat")
                        if projb_t is not None:
                            nc.vector.tensor_scalar_add(
                                sa_t, ps, projb_t[:, mt:mt + 1])
                        else:
                            nc.scalar.copy(out=sa_t, in_=ps)
                        nc.sync.dma_start(out=cc1_in[n][msl, :], in_=sa_t)
                    nc.gpsimd.collective_compute(
                        "AllReduce", ALU.add, replica_groups=groups,
                        ins=[cc1_in[n]], outs=[cc1_out[n]])

                for pi in range(NPAIR):
                    if pi % 2 == 0:
                        att_cur[0] = pool_att.tile([P, NRC, SLB], BF,
                                                   tag="attsl", name="attsl")
                    q0 = 2 * pi            # first q-block of pair
                    qsl = slice(q0 * P, (q0 + 2) * P)     # 256 queries
                    asl = slice((pi % 2) * 2 * P, (pi % 2 + 1) * 2 * P)
                    nkb = 2 * pi + 2
                    for h in range(HL):
                        hp, hr = h // 2, (h % 2) * 64
                        po = ps_o.tile([P, 2 * P], F32, tag="po")
                        for kb0 in range(0, nkb, 4):
                            kbn = min(4, nkb - kb0)
                            pss = ps_s.tile([P, 8 * P], F32, tag="pss")
                            for j in range(kbn):
                                kb = kb0 + j
                                jsl = slice(j * 2 * P, (j + 1) * 2 * P)
                                nc.tensor.matmul(
                                    pss[:, jsl],
                                    lhsT=kT[hr:hr + 64, hp, kb * P:(kb + 1) * P],
                                    rhs=qT[hr:hr + 64, hp, qsl],
                                    start=True, stop=True)
                                if kb == q0:
                                    nc.vector.tensor_add(
                                        pss[:, j * 2 * P:j * 2 * P + P],
                                        pss[:, j * 2 * P:j * 2 * P + P], maskT)
                                elif kb == q0 + 1:
                                    nc.vector.tensor_add(
                                        pss[:, j * 2 * P:j * 2 * P + P],
                                        pss[:, j * 2 * P:j * 2 * P + P],
                                        mask_full)
                                    nc.vector.tensor_add(
                                        pss[:, j * 2 * P + P:(j + 1) * 2 * P],
                                        pss[:, j * 2 * P + P:(j + 1) * 2 * P],
                                        maskT)
                            pexp = pool_p.tile([P, 8 * P], BF, tag="pexp")
                            nc.scalar.activation(out=pexp[:, 0:kbn * 2 * P],
                                                 in_=pss[:, 0:kbn * 2 * P],
                                                 func=AF.Exp,
                                                 scale=float(D) ** -0.5)
                            for j in range(kbn):
                                kb = kb0 + j
                                nc.tensor.matmul(
                                    po[0:D + 1, :],
                                    lhsT=vaug[:, kb, h, :],
                                    rhs=pexp[:, j * 2 * P:(j + 1) * 2 * P],
                                    start=(kb == 0), stop=(kb == nkb - 1))
                        # normalization fused into eviction
                        srow = pool_s.tile([1, 2 * P], F32, tag="srow")
                        nc.vector.tensor_copy(out=srow, in_=po[D:D + 1, :])
                        bc = pool_bc.tile([64, 2 * P], F32, tag="bc")
                        nc.gpsimd.partition_broadcast(out_ap=bc, in_ap=srow)
                        rc_ = pool_bc.tile([64, 2 * P], F32, tag="rc")
                        nc.vector.reciprocal_approx_fast(out=rc_, in_=bc)
                        nc.vector.tensor_mul(att_cur[0][hr:hr + 64, hp, asl],
                                             po[0:D, :], rc_)
                    if pi % 2 == 1:
                        proj_slab(pi // 2)
                for n in range(NSL):
                    emit_5b(n)

            mid.close()  # release qT/kT/vaug/att SBUF

            # ------- Back half: FFN + second AllReduce + final -------------
            with ExitStack() as bh:
                pool_fw = bh.enter_context(tc.tile_pool(name="fw", bufs=1))
                pool_g = bh.enter_context(tc.tile_pool(name="gT", bufs=2))
                pool_ev = bh.enter_context(tc.tile_pool(name="ev", bufs=3))
                pool_ffT = bh.enter_context(tc.tile_pool(name="ffT", bufs=2))
                pool_of = bh.enter_context(tc.tile_pool(name="of", bufs=2))
                ps_f = bh.enter_context(
                    tc.tile_pool(name="psf", bufs=3, space="PSUM"))
                ps_f2 = bh.enter_context(
                    tc.tile_pool(name="psf2", bufs=2, space="PSUM"))

                def emit_final(k):
                    ffT = pool_ffT.tile([P, NCC, SLB], BF, tag="ffT")
                    for kc in range(NCC):
                        nc.sync.dma_start(
                            out=ffT[:, kc, :],
                            in_=cc2_out[k][kc * P:(kc + 1) * P, :])
                    for itl in range(SLB // P):
                        it = k * (SLB // P) + itl
                        sl = slice(it * P, (it + 1) * P)
                        lsl = slice(itl * P, (itl + 1) * P)
                        ptf = ps_tr1.tile([P, C], BF, tag="p1k")
                        for kc in range(NCC):
                            nc.tensor.transpose(ptf[:, kc * P:(kc + 1) * P],
                                                ffT[:, kc, lsl], ident)
                        xt = pool_xs.tile([P, C], F32, tag="xt2")
                        nc.sync.dma_start(out=xt, in_=x2_dram[k][lsl, :])
                        ot = pool_of.tile([P, C], F32, tag="of")
                        nc.vector.tensor_add(ot, xt, ptf)
                        nc.sync.dma_start(out=out_d[sl, :], in_=ot)

                f1w_sb = load_chunks(pool_fw, f1_d, NCC, FH, "f1w")
                f2w_sb = load_chunks(pool_fw, f2_d, FH // P, C, "f2w")

                for n in range(NSL):
                    h2T = h2T_sl[n]
                    gT = pool_g.tile([P, FH // P, SLB], BF, tag="gT")

                    # f1 for this slab
                    for m in range(FH // P):
                        ps = ps_f.tile([P, SLB], F32, tag="psf1")
                        for kc in range(NCC):
                            nc.tensor.matmul(ps,
                                             lhsT=f1w_sb[:, kc, m * P:(m + 1) * P],
                                             rhs=h2T[:, kc, :],
                                             start=(kc == 0),
                                             stop=(kc == NCC - 1))
                        if f1b_t is not None:
                            nc.scalar.activation(out=gT[:, m, :], in_=ps,
                                                 func=AF.Gelu,
                                                 bias=f1b_t[:, m:m + 1],
                                                 scale=1.0)
                        else:
                            nc.scalar.activation(out=gT[:, m, :], in_=ps,
                                                 func=AF.Gelu, scale=1.0)

                    if n >= 1:
                        emit_final(n - 1)

                    # f2 for this slab
                    for mt in range(NCC):
                        msl = slice(mt * P, (mt + 1) * P)
                        ps = ps_f2.tile([P, SLB], F32, tag="psf2")
                        for kf in range(FH // P):
                            nc.tensor.matmul(ps,
                                             lhsT=f2w_sb[:, kf, msl],
                                             rhs=gT[:, kf, :],
                                             start=(kf == 0),
                                             stop=(kf == FH // P - 1))
                        ev = pool_ev.tile([P, SLB], BF, tag="ffev")
                        if f2b_t is not None:
                            nc.scalar.activation(out=ev, in_=ps, func=AF.Identity,
                                                 bias=f2b_t[:, mt:mt + 1],
                                                 scale=1.0)
                        else:
                            nc.scalar.copy(out=ev, in_=ps)
                        nc.sync.dma_start(out=cc2_in[n][msl, :], in_=ev)
                    nc.gpsimd.collective_compute(
                        "AllReduce", ALU.add, replica_groups=groups,
                        ins=[cc2_in[n]], outs=[cc2_out[n]])

                emit_final(NSL - 1)

    nc.compile()
    return nc


def kernel(**inputs):
    x = np.asarray(inputs["x"], dtype=np.float32)
    q_w = np.asarray(inputs["q_w"], dtype=np.float32)
    kvd_w = np.asarray(inputs["kvd_w"], dtype=np.float32)
    kvu_w = np.asarray(inputs["kvu_w"], dtype=np.float32)
    proj_w = np.asarray(inputs["proj_w"], dtype=np.float32)
    f1_w = np.asarray(inputs["f1_w"], dtype=np.float32)
    f2_w = np.asarray(inputs["f2_w"], dtype=np.float32)
    ln1_w = np.asarray(inputs["ln1_w"], dtype=np.float32)
    ln1_b = np.asarray(inputs["ln1_b"], dtype=np.float32)
    ln2_w = np.asarray(inputs["ln2_w"], dtype=np.float32)
    ln2_b = np.asarray(inputs["ln2_b"], dtype=np.float32)
    kvln_w = np.asarray(inputs["kvln_w"], dtype=np.float32)
    kvln_b = np.asarray(inputs["kvln_b"], dtype=np.float32)
    proj_b = np.asarray(inputs["proj_b"], dtype=np.float32)
    f1_b = np.asarray(inputs["f1_b"], dtype=np.float32)
    f2_b = np.asarray(inputs["f2_b"], dtype=np.float32)

    flags = (bool(np.allclose(ln1_w, 1) and np.allclose(ln1_b, 0)),
             bool(np.allclose(kvln_w, 1) and np.allclose(kvln_b, 0)),
             bool(np.allclose(ln2_w, 1) and np.allclose(ln2_b, 0)),
             bool(np.allclose(proj_b, 0)),
             bool(np.allclose(f1_b, 0)),
             bool(np.allclose(f2_b, 0)))
    if flags not in _CACHE:
        _CACHE[flags] = _build(flags)
    nc = _CACHE[flags]

    cos, sinf = _rope_tables()
    kvu_v4 = kvu_w.reshape(R, 2, H, D)
    in_maps = []
    for c in range(8):
        b, half = c // 2, c % 2
        hsl = slice(half * HL, (half + 1) * HL)
        m = {
            "x_loc": np.ascontiguousarray(x[b]),
            "qw_loc": np.ascontiguousarray(
                q_w[:, half * HD:(half + 1) * HD]).astype(BF16),
            "kvd_w": kvd_w.astype(BF16),
            "kvu_k": np.ascontiguousarray(
                kvu_v4[:, 0, hsl, :].reshape(R, HD)).astype(BF16),
            "kvu_v": np.ascontiguousarray(
                kvu_v4[:, 1, hsl, :].reshape(R, HD)).astype(BF16),
            "proj_w_loc": np.ascontiguousarray(
                proj_w[half * HD:(half + 1) * HD, :]).astype(BF16),
            "f1_w_loc": np.ascontiguousarray(
                f1_w[:, half * FH:(half + 1) * FH]).astype(BF16),
            "f2_w_loc": np.ascontiguousarray(
                f2_w[half * FH:(half + 1) * FH, :]).astype(BF16),
            "cos_t": cos,
            "sinf_t": sinf,
        }
        if not flags[0]:
            m["ln1_w"], m["ln1_b"] = ln1_w, ln1_b
        if not flags[1]:
            m["kvln_w"], m["kvln_b"] = kvln_w, kvln_b
        if not flags[2]:
            m["ln2_w"], m["ln2_b"] = ln2_w, ln2_b
        if not flags[3]:
            m["proj_b"] = proj_b
        if not flags[4]:
            m["f1_b_loc"] = np.ascontiguousarray(f1_b[half * FH:(half + 1) * FH])
        if not flags[5]:
            m["f2_b"] = f2_b
        in_maps.append(m)

    res = run_bass_kernel_spmd(nc, in_maps, list(range(8)), trace=TRACE)
    kernel.last_result = res
    out = np.stack([res.results[2 * b]["out_loc"] for b in range(B)])
    return out


# revision 31
# speedup vs baseline: 1.0067x; 1.0067x over previous
"""Trainium2 Bass kernel for nn_Block_11897059410591 (MLA transformer block).

Sharding over 8 NeuronCores: core c = (batch b=c//2, head-half h0=(c%2)*8).
Each core computes LN1/kvd/kvu/RoPE for its whole batch, causal attention for
its 8 heads, a partial output projection (contracted over its heads) that is
pair-AllReduced, then the FFN with d_ff split in half across the pair and a
second pair-AllReduce. Both cores of a pair end with the identical full-batch
output; the host keeps the even core's copy. The back half (proj -> residual
-> FFN -> output) is pipelined over 4 token slabs so the AllReduces overlap
with compute.
"""
import sys

if "/opt/trn_rl_repo" not in sys.path:
    sys.path.insert(0, "/opt/trn_rl_repo")

import numpy as np
import ml_dtypes


def _ensure_ntff_hook():
    """antenv.axon_hooks is missing in this image; shim it so
    run_bass_kernel_spmd(trace=True) can capture NTFF profiles."""
    try:
        from antenv import axon_hooks  # noqa: F401
        return
    except ImportError:
        pass
    try:
        import types
        import importlib.util
        m = types.ModuleType("antenv.axon_hooks")
        _hook = [None]
        m.set_axon_ntff_profile_hook = lambda h: _hook.__setitem__(0, h)
        m.get_axon_ntff_profile_hook = lambda: _hook[0]
        sys.modules["antenv.axon_hooks"] = m
        import antenv
        antenv.axon_hooks = m
        spec = importlib.util.spec_from_file_location(
            "_trn_boot_shim", "/root/.axon_site/trn_agent_boot/trn_boot.py")
        tb = importlib.util.module_from_spec(spec)
        spec.loader.exec_module(tb)
        hook = tb._ntff_profile_via_ctypes("/opt/axon/libaxon_pjrt.so")
        m.set_axon_ntff_profile_hook(hook)
    except Exception as e:  # degrade to trace-less operation
        print(f"ntff hook shim failed ({e}); tracing disabled", file=sys.stderr)


_ensure_ntff_hook()

import concourse.bass as bass
import concourse.mybir as mybir
import concourse.tile as tile
from concourse import bacc
from concourse.bass_utils import run_bass_kernel_spmd
from concourse.masks import make_identity

F32 = mybir.dt.float32
BF = mybir.dt.bfloat16
BF16 = ml_dtypes.bfloat16
AF = mybir.ActivationFunctionType
ALU = mybir.AluOpType

B, T, C = 4, 2048, 1024
H, D, R, FF = 16, 64, 512, 4096
HL = 8              # heads per core
HD = HL * D         # 512
FH = FF // 2        # 2048, d_ff half per core
P = 128
NT = T // P         # 16 token chunks
NCC = C // P        # 8 C chunks
NRC = R // P        # 4 R chunks
NSL = 4             # token slabs for the back half
SLB = T // NSL      # 512 tokens per slab
LN_EPS = 1e-5

TRACE = False
_CACHE = {}


def _rope_tables():
    inv_freq = 1.0 / (10000.0 ** (np.arange(0, D, 2, dtype=np.float32) / D))
    t = np.arange(T, dtype=np.float32)
    freqs = np.outer(t, inv_freq)
    emb = np.concatenate([freqs, freqs], axis=-1)  # [T, D]
    cos = np.cos(emb).astype(np.float32)
    sin = np.sin(emb).astype(np.float32)
    sinf = sin.copy()
    sinf[:, : D // 2] = -sinf[:, : D // 2]
    return cos, sinf


def _build(flags):
    (ln1_triv, kvln_triv, ln2_triv, pb0, f1b0, f2b0) = flags
    nc = bacc.Bacc("TRN2", target_bir_lowering=False, debug=False,
                   enable_asserts=False, num_devices=8)

    x_d = nc.dram_tensor("x_loc", [T, C], F32, kind="ExternalInput").ap()
    qw_d = nc.dram_tensor("qw_loc", [C, HD], BF, kind="ExternalInput").ap()
    kvd_d = nc.dram_tensor("kvd_w", [C, R], BF, kind="ExternalInput").ap()
    kvuk_d = nc.dram_tensor("kvu_k", [R, HD], BF, kind="ExternalInput").ap()
    kvuv_d = nc.dram_tensor("kvu_v", [R, HD], BF, kind="ExternalInput").ap()
    pw_d = nc.dram_tensor("proj_w_loc", [HD, C], BF, kind="ExternalInput").ap()
    f1_d = nc.dram_tensor("f1_w_loc", [C, FH], BF, kind="ExternalInput").ap()
    f2_d = nc.dram_tensor("f2_w_loc", [FH, C], BF, kind="ExternalInput").ap()
    cos_d = nc.dram_tensor("cos_t", [T, D], F32, kind="ExternalInput").ap()
    sinf_d = nc.dram_tensor("sinf_t", [T, D], F32, kind="ExternalInput").ap()
    out_d = nc.dram_tensor("out_loc", [T, C], F32, kind="ExternalOutput").ap()

    opt_ins = {}
    if not ln1_triv:
        opt_ins["ln1_w"] = nc.dram_tensor("ln1_w", [C], F32, kind="ExternalInput").ap()
        opt_ins["ln1_b"] = nc.dram_tensor("ln1_b", [C], F32, kind="ExternalInput").ap()
    if not kvln_triv:
        opt_ins["kvln_w"] = nc.dram_tensor("kvln_w", [R], F32, kind="ExternalInput").ap()
        opt_ins["kvln_b"] = nc.dram_tensor("kvln_b", [R], F32, kind="ExternalInput").ap()
    if not ln2_triv:
        opt_ins["ln2_w"] = nc.dram_tensor("ln2_w", [C], F32, kind="ExternalInput").ap()
        opt_ins["ln2_b"] = nc.dram_tensor("ln2_b", [C], F32, kind="ExternalInput").ap()
    if not pb0:
        opt_ins["proj_b"] = nc.dram_tensor("proj_b", [C], F32, kind="ExternalInput").ap()
    if not f1b0:
        opt_ins["f1_b"] = nc.dram_tensor("f1_b_loc", [FH], F32, kind="ExternalInput").ap()
    if not f2b0:
        opt_ins["f2_b"] = nc.dram_tensor("f2_b", [C], F32, kind="ExternalInput").ap()

    # internal DRAM
    cc1_in = [nc.dram_tensor(f"cc1_in{n}", [C, SLB], BF).ap()
              for n in range(NSL)]
    cc1_out = [nc.dram_tensor(f"cc1_out{n}", [C, SLB], BF).ap()
               for n in range(NSL)]
    cc2_in = [nc.dram_tensor(f"cc2_in{n}", [C, SLB], BF).ap()
              for n in range(NSL)]
    cc2_out = [nc.dram_tensor(f"cc2_out{n}", [C, SLB], BF).ap()
               for n in range(NSL)]
    x2_dram = [nc.dram_tensor(f"x2_bounce{n}", [SLB, C], F32).ap()
               for n in range(NSL)]
    groups = [[0, 1], [2, 3], [4, 5], [6, 7]]

    def bcast_free(ap2d, n, width):
        """[P, width] AP -> [P, n, width] with 0-step middle dim."""
        return bass.AP(tensor=ap2d.tensor, offset=ap2d.offset,
                       ap=[ap2d.ap[0], [0, n], [1, width]])

    def ln_stats(pool, src_ap, width, eps_t):
        """Per-partition (mean, rstd) of src_ap [P, width]."""
        ngr = (width + 511) // 512
        st6 = pool.tile([P, ngr, 6], F32, tag="st6")
        sv = src_ap.rearrange("p (g d) -> p g d", g=ngr)
        for g in range(ngr):
            nc.vector.bn_stats(out=st6[:, g, :], in_=sv[:, g, :])
        mv = pool.tile([P, 2], F32, tag="mv")
        nc.vector.bn_aggr(out=mv, in_=st6)
        nc.scalar.activation(out=mv[:, 1:2], in_=mv[:, 1:2], func=AF.Sqrt,
                             bias=eps_t, scale=1.0)
        nc.vector.reciprocal(out=mv[:, 1:2], in_=mv[:, 1:2])
        return mv

    from contextlib import ExitStack
    with tile.TileContext(nc) as tc:
        with ExitStack() as ctx:
            const = ctx.enter_context(tc.tile_pool(name="const", bufs=1))
            ident = const.tile([P, P], BF)
            make_identity(nc, ident)
            eps_t = const.tile([P, 1], F32)
            nc.vector.memset(eps_t, LN_EPS)
            # S^T diagonal causal mask: keep (0) where col(q) >= row(k)
            maskT = const.tile([P, P], F32)
            nc.gpsimd.memset(maskT, 0.0)
            nc.gpsimd.affine_select(out=maskT, in_=maskT, compare_op=ALU.is_ge,
                                    fill=-1e9, base=0, pattern=[[1, P]],
                                    channel_multiplier=-1)
            mask_full = const.tile([P, P], F32)
            nc.vector.memset(mask_full, -1e9)

            def dram_row_bcast(name, ap1d, width):
                t = const.tile([P, width], F32, name=name)
                src = bass.AP(tensor=ap1d.tensor, offset=ap1d.offset,
                              ap=[[0, P], [1, width]])
                nc.sync.dma_start(out=t, in_=src)
                return t

            ln1_wt = ln1_bt = ln2_wt = ln2_bt = kvln_wt = kvln_bt = None
            if not ln1_triv:
                ln1_wt = dram_row_bcast("ln1w_b", opt_ins["ln1_w"], C)
                ln1_bt = dram_row_bcast("ln1b_b", opt_ins["ln1_b"], C)
            if not kvln_triv:
                kvln_wt = dram_row_bcast("kvlnw_b", opt_ins["kvln_w"], R)
                kvln_bt = dram_row_bcast("kvlnb_b", opt_ins["kvln_b"], R)
            if not ln2_triv:
                ln2_wt = dram_row_bcast("ln2w_b", opt_ins["ln2_w"], C)
                ln2_bt = dram_row_bcast("ln2b_b", opt_ins["ln2_b"], C)
            projb_t = f1b_t = f2b_t = None
            if not pb0:
                projb_t = const.tile([P, NCC], F32, name="projb")
                nc.sync.dma_start(out=projb_t, in_=opt_ins["proj_b"].rearrange(
                    "(m p) -> p m", p=P))
            if not f1b0:
                f1b_t = const.tile([P, FH // P], F32, name="f1b")
                nc.sync.dma_start(out=f1b_t, in_=opt_ins["f1_b"].rearrange(
                    "(m p) -> p m", p=P))
            if not f2b0:
                f2b_t = const.tile([P, NCC], F32, name="f2b")
                nc.sync.dma_start(out=f2b_t, in_=opt_ins["f2_b"].rearrange(
                    "(m p) -> p m", p=P))

            def load_chunks(pool, dram_ap, nk, width, name):
                t = pool.tile([P, nk, width], BF, name=name)
                for k in range(nk):
                    nc.sync.dma_start(out=t[:, k, :],
                                      in_=dram_ap[k * P:(k + 1) * P, :])
                return t

            # ---- long-lived pools (created early; closed last, LIFO) ------
            pool_xs = ctx.enter_context(tc.tile_pool(name="xs2", bufs=2))
            pool_h2T = ctx.enter_context(tc.tile_pool(name="h2T", bufs=4))
            ps_tr1 = ctx.enter_context(
                tc.tile_pool(name="pstr1", bufs=2, space="PSUM"))
            h2T_sl = [None] * NSL
            fivb_pools = {}

            def emit_5b(n):
                """Residual + LN2 + h2T for token slab n."""
                h2T = pool_h2T.tile([P, NCC, SLB], BF, tag="h2T")
                h2T_sl[n] = h2T
                saT = fivb_pools["saT"].tile([P, NCC, SLB], BF, tag="saT",
                                             name="saT")
                for kc in range(NCC):
                    nc.sync.dma_start(
                        out=saT[:, kc, :],
                        in_=cc1_out[n][kc * P:(kc + 1) * P, :])
                for itl in range(SLB // P):
                    it = n * (SLB // P) + itl
                    sl = slice(it * P, (it + 1) * P)
                    lsl = slice(itl * P, (itl + 1) * P)
                    pt1 = ps_tr1.tile([P, C], BF, tag="p1k")
                    for kc in range(NCC):
                        nc.tensor.transpose(pt1[:, kc * P:(kc + 1) * P],
                                            saT[:, kc, lsl], ident)
                    xt = pool_xs.tile([P, C], F32, tag="xt2")
                    nc.sync.dma_start(out=xt, in_=x_d[sl, :])
                    x2t = fivb_pools["x2"].tile([P, C], F32, tag="x2t",
                                                name="x2t")
                    nc.vector.tensor_add(x2t, xt, pt1)
                    nc.sync.dma_start(out=x2_dram[n][lsl, :], in_=x2t)
                    mv = ln_stats(fivb_pools["st2"], x2t, C, eps_t)
                    h2 = fivb_pools["tok2"].tile([P, C], BF, tag="h2", name="h2")
                    nc.vector.tensor_scalar(out=h2, in0=x2t,
                                            scalar1=mv[:, 0:1],
                                            scalar2=mv[:, 1:2],
                                            op0=ALU.subtract, op1=ALU.mult)
                    if ln2_wt is not None:
                        nc.vector.tensor_mul(h2, h2, ln2_wt)
                        nc.vector.tensor_add(h2, h2, ln2_bt)
                    pt2 = ps_tr1.tile([P, C], BF, tag="p1k")
                    for kc in range(NCC):
                        nc.tensor.transpose(pt2[:, kc * P:(kc + 1) * P],
                                            h2[:, kc * P:(kc + 1) * P],
                                            ident)
                    nc.scalar.copy(
                        out=h2T[:, :, lsl],
                        in_=pt2.rearrange("p (kc t) -> p kc t", kc=NCC))

            # ---------------- qkv/att scope --------------------------------
            mid = ExitStack()
            pool_qkv = mid.enter_context(tc.tile_pool(name="qkv", bufs=1))
            pool_att = mid.enter_context(tc.tile_pool(name="att", bufs=2))
            qT = pool_qkv.tile([P, HL // 2, T], BF)   # [(2h,64d), hp, T]
            kT = pool_qkv.tile([P, HL // 2, T], BF)
            vaug = pool_qkv.tile([P, NT, HL, D + 1], BF)

            # ------- fused prep: LN1 / h^T / ckv^T / q / k / v per chunk ---
            with ExitStack() as prep:
                pool_xsp = prep.enter_context(tc.tile_pool(name="xs", bufs=3))
                pool_stp = prep.enter_context(tc.tile_pool(name="st", bufs=4))
                pool_tokp = prep.enter_context(tc.tile_pool(name="tok", bufs=3))
                pool_w = prep.enter_context(tc.tile_pool(name="wts", bufs=1))
                pool_cs = prep.enter_context(tc.tile_pool(name="cs", bufs=3))
                pool_ro = prep.enter_context(tc.tile_pool(name="ro", bufs=3))
                pool_hts = prep.enter_context(tc.tile_pool(name="hts", bufs=4))
                pool_cks = prep.enter_context(tc.tile_pool(name="cks", bufs=4))
                ps_big = prep.enter_context(
                    tc.tile_pool(name="psbig", bufs=3, space="PSUM"))
                ps_tr = prep.enter_context(
                    tc.tile_pool(name="pstr", bufs=3, space="PSUM"))

                qw_sb = load_chunks(pool_w, qw_d, NCC, HD, "qw")
                kvdw_sb = load_chunks(pool_w, kvd_d, NCC, R, "kvdw")
                kvuk_sb = load_chunks(pool_w, kvuk_d, NRC, HD, "kvuk")
                kvuv_sb = load_chunks(pool_w, kvuv_d, NRC, HD, "kvuv")

                for it in range(NT):
                    sl = slice(it * P, (it + 1) * P)
                    xt = pool_xsp.tile([P, C], F32)
                    nc.sync.dma_start(out=xt, in_=x_d[sl, :])
                    mv = ln_stats(pool_stp, xt, C, eps_t)
                    ht = pool_tokp.tile([P, C], BF, tag="ht")
                    nc.vector.tensor_scalar(out=ht, in0=xt,
                                            scalar1=mv[:, 0:1], scalar2=mv[:, 1:2],
                                            op0=ALU.subtract, op1=ALU.mult)
                    if ln1_wt is not None:
                        nc.vector.tensor_mul(ht, ht, ln1_wt)
                        nc.vector.tensor_add(ht, ht, ln1_bt)
                    hTt = pool_hts.tile([P, NCC, P], BF, tag="hTt")
                    for kc in range(NCC):
                        pt = ps_tr.tile([P, P], BF, tag="ptr")
                        nc.tensor.transpose(pt, ht[:, kc * P:(kc + 1) * P], ident)
                        if kc % 2 == 0:
                            nc.scalar.copy(out=hTt[:, kc, :], in_=pt)
                        else:
                            nc.vector.tensor_copy(out=hTt[:, kc, :], in_=pt)

                    # ckv = LN(h @ kvd_w), transposed
                    ps = ps_big.tile([P, R], F32, tag="psb")
                    for kc in range(NCC):
                        nc.tensor.matmul(ps, lhsT=hTt[:, kc, :],
                                         rhs=kvdw_sb[:, kc, :],
                                         start=(kc == 0), stop=(kc == NCC - 1))
                    mv = ln_stats(pool_stp, ps, R, eps_t)
                    ct = pool_tokp.tile([P, R], BF, tag="ckvtok")
                    nc.vector.tensor_scalar(out=ct, in0=ps,
                                            scalar1=mv[:, 0:1], scalar2=mv[:, 1:2],
                                            op0=ALU.subtract, op1=ALU.mult)
                    if kvln_wt is not None:
                        nc.vector.tensor_mul(ct, ct, kvln_wt)
                        nc.vector.tensor_add(ct, ct, kvln_bt)
                    ckvTt = pool_cks.tile([P, NRC, P], BF, tag="ckvTt")
                    for rc in range(NRC):
                        pt = ps_tr.tile([P, P], BF, tag="ptr")
                        nc.tensor.transpose(pt, ct[:, rc * P:(rc + 1) * P], ident)
                        if rc % 2 == 0:
                            nc.scalar.copy(out=ckvTt[:, rc, :], in_=pt)
                        else:
                            nc.vector.tensor_copy(out=ckvTt[:, rc, :], in_=pt)

                    cos_sb = pool_cs.tile([P, D], F32, tag="cos")
                    nc.sync.dma_start(out=cos_sb, in_=cos_d[sl, :])
                    sinf_sb = pool_cs.tile([P, D], F32, tag="sinf")
                    nc.sync.dma_start(out=sinf_sb, in_=sinf_d[sl, :])

                    for which in ("q", "k"):
                        ps = ps_big.tile([P, HD], F32, tag="psb")
                        if which == "q":
                            for kc in range(NCC):
                                nc.tensor.matmul(ps, lhsT=hTt[:, kc, :],
                                                 rhs=qw_sb[:, kc, :],
                                                 start=(kc == 0),
                                                 stop=(kc == NCC - 1))
                        else:
                            for rc in range(NRC):
                                nc.tensor.matmul(ps, lhsT=ckvTt[:, rc, :],
                                                 rhs=kvuk_sb[:, rc, :],
                                                 start=(rc == 0),
                                                 stop=(rc == NRC - 1))
                        psv = ps.rearrange("p (h d) -> p h d", d=D)
                        t1 = pool_ro.tile([P, HL, D], BF, tag="t1")
                        nc.vector.tensor_mul(t1, psv, bcast_free(cos_sb, HL, D))
                        t2 = pool_ro.tile([P, HL, D], BF, tag="t2")
                        half = D // 2
                        sfv = sinf_sb
                        nc.vector.tensor_mul(
                            t2[:, :, 0:half],
                            bass.AP(tensor=psv.tensor, offset=psv.offset + half,
                                    ap=[psv.ap[0], [D, HL], [1, half]]),
                            bass.AP(tensor=sfv.tensor, offset=sfv.offset,
                                    ap=[sfv.ap[0], [0, HL], [1, half]]))
                        nc.vector.tensor_mul(
                            t2[:, :, half:D],
                            bass.AP(tensor=psv.tensor, offset=psv.offset,
                                    ap=[psv.ap[0], [D, HL], [1, half]]),
                            bass.AP(tensor=sfv.tensor, offset=sfv.offset + half,
                                    ap=[sfv.ap[0], [0, HL], [1, half]]))
                        ro = pool_ro.tile([P, HL, D], BF, tag="ro")
                        nc.vector.tensor_add(ro, t1, t2)
                        dstT = qT if which == "q" else kT
                        for h in range(HL):
                            hp, hr = h // 2, (h % 2) * 64
                            pt = ps_tr.tile([64, P], BF, tag="ptr")
                            nc.tensor.transpose(pt, ro[:, h, :], ident)
                            if h % 2 == 0:
                                nc.scalar.copy(out=dstT[0:64, hp, sl], in_=pt)
                            else:
                                nc.vector.tensor_copy(out=dstT[64:128, hp, sl],
                                                      in_=pt)

                    # v (no rope) -> token-major vaug
                    ps = ps_big.tile([P, HD], F32, tag="psb")
                    for rc in range(NRC):
                        nc.tensor.matmul(ps, lhsT=ckvTt[:, rc, :],
                                         rhs=kvuv_sb[:, rc, :],
                                         start=(rc == 0), stop=(rc == NRC - 1))
                    nc.vector.memset(vaug[:, it, :, D:D + 1], 1.0)
                    nc.scalar.copy(out=vaug[:, it, :, 0:D],
                                   in_=ps.rearrange("p (h d) -> p h d", d=D))

            # ---- Phase 3+5a+5b: attention / proj / residual interleaved ---
            with ExitStack() as attn:
                pool_p = attn.enter_context(tc.tile_pool(name="pexp", bufs=3))
                pool_s = attn.enter_context(tc.tile_pool(name="srow", bufs=4))
                pool_bc = attn.enter_context(tc.tile_pool(name="bc", bufs=4))
                pool_pw = attn.enter_context(tc.tile_pool(name="pw", bufs=1))
                pool_sa = attn.enter_context(tc.tile_pool(name="sa", bufs=3))
                ps_s = attn.enter_context(
                    tc.tile_pool(name="pss", bufs=2, space="PSUM"))
                ps_o = attn.enter_context(
                    tc.tile_pool(name="pso", bufs=2, space="PSUM"))
                pw_sb = load_chunks(pool_pw, pw_d, NRC, C, "pw")
                NPAIR = NT // 2
                att_cur = [None]  # per-slab [(2h,64d), hp, SLB] tile

                def proj_slab(n):
                    att = att_cur[0]
                    for mt in range(NCC):
                        msl = slice(mt * P, (mt + 1) * P)
                        ps = ps_s.tile([P, SLB], F32, tag="pss")
                        for kc in range(NRC):
                            nc.tensor.matmul(ps, lhsT=pw_sb[:, kc, msl],
                                             rhs=att[:, kc, :],
                                             start=(kc == 0),
                                             stop=(kc == NRC - 1))
                        sa_t = pool_sa.tile([P, SLB], BF, tag="sat")
                        if projb_t is not None:
                            nc.vector.tensor_scalar_add(
                                sa_t, ps, projb_t[:, mt:mt + 1])
                        else:
                            nc.scalar.copy(out=sa_t, in_=ps)
                        nc.sync.dma_start(out=cc1_in[n][msl, :], in_=sa_t)
                    nc.gpsimd.collective_compute(
                        "AllReduce", ALU.add, replica_groups=groups,
                        ins=[cc1_in[n]], outs=[cc1_out[n]])

                for pi in range(NPAIR):
                    if pi % 2 == 0:
                        att_cur[0] = pool_att.tile([P, NRC, SLB], BF,
                                                   tag="attsl", name="attsl")
                    q0 = 2 * pi            # first q-block of pair
                    qsl = slice(q0 * P, (q0 + 2) * P)     # 256 queries
                    asl = slice((pi % 2) * 2 * P, (pi % 2 + 1) * 2 * P)
                    nkb = 2 * pi + 2
                    for h in range(HL):
                        hp, hr = h // 2, (h % 2) * 64
                        po = ps_o.tile([P, 2 * P], F32, tag="po")
                        for kb0 in range(0, nkb, 4):
                            kbn = min(4, nkb - kb0)
                            pss = ps_s.tile([P, 8 * P], F32, tag="pss")
                            for j in range(kbn):
                                kb = kb0 + j
                                jsl = slice(j * 2 * P, (j + 1) * 2 * P)
                                nc.tensor.matmul(
                                    pss[:, jsl],
                                    lhsT=kT[hr:hr + 64, hp, kb * P:(kb + 1) * P],
                                    rhs=qT[hr:hr + 64, hp, qsl],
                                    start=True, stop=True)
                                if kb == q0:
                                    nc.vector.tensor_add(
                                        pss[:, j * 2 * P:j * 2 * P + P],
                                        pss[:, j * 2 * P:j * 2 * P + P], maskT)
                                elif kb == q0 + 1:
                                    nc.vector.tensor_add(
                                        pss[:, j * 2 * P:j * 2 * P + P],
                                        pss[:, j * 2 * P:j * 2 * P + P],
                                        mask_full)
                                    nc.vector.tensor_add(
                                        pss[:, j * 2 * P + P:(j + 1) * 2 * P],
                                        pss[:, j * 2 * P + P:(j + 1) * 2 * P],
                                        maskT)
                            pexp = pool_p.tile([P, 8 * P], BF, tag="pexp")
                            nc.scalar.activation(out=pexp[:, 0:kbn * 2 * P],
                                                 in_=pss[:, 0:kbn * 2 * P],
                                                 func=AF.Exp,
                                                 scale=float(D) ** -0.5)
                            for j in range(kbn):
                                kb = kb0 + j
                                nc.tensor.matmul(
                                    po[0:D + 1, :],
                                    lhsT=vaug[:, kb, h, :],
                                    rhs=pexp[:, j * 2 * P:(j + 1) * 2 * P],
                                    start=(kb == 0), stop=(kb == nkb - 1))
                        # normalization fused into eviction
                        srow = pool_s.tile([1, 2 * P], F32, tag="srow")
                        nc.vector.tensor_copy(out=srow, in_=po[D:D + 1, :])
                        bc = pool_bc.tile([64, 2 * P], F32, tag="bc")
                        nc.gpsimd.partition_broadcast(out_ap=bc, in_ap=srow)
                        rc_ = pool_bc.tile([64, 2 * P], F32, tag="rc")
                        nc.vector.reciprocal_approx_fast(out=rc_, in_=bc)
                        nc.vector.tensor_mul(att_cur[0][hr:hr + 64, hp, asl],
                                             po[0:D, :], rc_)
                    if pi % 2 == 1:
                        proj_slab(pi // 2)

            mid.close()  # release qT/kT/vaug/att SBUF

            # ------- Back half: FFN + second AllReduce + final -------------
            with ExitStack() as bh:
                pool_fw = bh.enter_context(tc.tile_pool(name="fw", bufs=1))
                pool_g = bh.enter_context(tc.tile_pool(name="gT", bufs=2))
                pool_ev = bh.enter_context(tc.tile_pool(name="ev", bufs=2))
                pool_ffT = bh.enter_context(tc.tile_pool(name="ffT", bufs=2))
                pool_of = bh.enter_context(tc.tile_pool(name="of", bufs=2))
                ps_f = bh.enter_context(
                    tc.tile_pool(name="psf", bufs=3, space="PSUM"))
                ps_f2 = bh.enter_context(
                    tc.tile_pool(name="psf2", bufs=2, space="PSUM"))
                fivb_pools["saT"] = bh.enter_context(
                    tc.tile_pool(name="saT", bufs=2))
                fivb_pools["x2"] = bh.enter_context(
                    tc.tile_pool(name="x2", bufs=2))
                fivb_pools["st2"] = bh.enter_context(
                    tc.tile_pool(name="st2", bufs=4))
                fivb_pools["tok2"] = bh.enter_context(
                    tc.tile_pool(name="tok2", bufs=2))

                def emit_final(k):
                    ffT = pool_ffT.tile([P, NCC, SLB], BF, tag="ffT")
                    for kc in range(NCC):
                        nc.sync.dma_start(
                            out=ffT[:, kc, :],
                            in_=cc2_out[k][kc * P:(kc + 1) * P, :])
                    for itl in range(SLB // P):
                        it = k * (SLB // P) + itl
                        sl = slice(it * P, (it + 1) * P)
                        lsl = slice(itl * P, (itl + 1) * P)
                        ptf = ps_tr1.tile([P, C], BF, tag="p1k")
                        for kc in range(NCC):
                            nc.tensor.transpose(ptf[:, kc * P:(kc + 1) * P],
                                                ffT[:, kc, lsl], ident)
                        xt = pool_xs.tile([P, C], F32, tag="xt2")
                        nc.sync.dma_start(out=xt, in_=x2_dram[k][lsl, :])
                        ot = pool_of.tile([P, C], F32, tag="of")
                        nc.vector.tensor_add(ot, xt, ptf)
                        nc.sync.dma_start(out=out_d[sl, :], in_=ot)

                f1w_sb = load_chunks(pool_fw, f1_d, NCC, FH, "f1w")
                f2w_sb = load_chunks(pool_fw, f2_d, FH // P, C, "f2w")

                for n in range(NSL):
                    emit_5b(n)
                    h2T = h2T_sl[n]
                    gT = pool_g.tile([P, FH // P, SLB], BF, tag="gT")

                    # f1 for this slab
                    for m in range(FH // P):
                        ps = ps_f.tile([P, SLB], F32, tag="psf1")
                        for kc in range(NCC):
                            nc.tensor.matmul(ps,
                                             lhsT=f1w_sb[:, kc, m * P:(m + 1) * P],
                                             rhs=h2T[:, kc, :],
                                             start=(kc == 0),
                                             stop=(kc == NCC - 1))
                        if f1b_t is not None:
                            nc.scalar.activation(out=gT[:, m, :], in_=ps,
                                                 func=AF.Gelu,
                                                 bias=f1b_t[:, m:m + 1],
                                                 scale=1.0)
                        else:
                            nc.scalar.activation(out=gT[:, m, :], in_=ps,
                                                 func=AF.Gelu, scale=1.0)

                    if n >= 1:
                        emit_final(n - 1)

                    # f2 for this slab
                    for mt in range(NCC):
                        msl = slice(mt * P, (mt + 1) * P)
                        ps = ps_f2.tile([P, SLB], F32, tag="psf2")
                        for kf in range(FH // P):
                            nc.tensor.matmul(ps,
                                             lhsT=f2w_sb[:, kf, msl],
                                             rhs=gT[:, kf, :],
                                             start=(kf == 0),
                                             stop=(kf == FH // P - 1))
                        ev = pool_ev.tile([P, SLB], BF, tag="ffev")
                        if f2b_t is not None:
                            nc.scalar.activation(out=ev, in_=ps, func=AF.Identity,
                                                 bias=f2b_t[:, mt:mt + 1],
                                                 scale=1.0)
                        else:
                            nc.scalar.copy(out=ev, in_=ps)
                        nc.sync.dma_start(out=cc2_in[n][msl, :], in_=ev)
                    nc.gpsimd.collective_compute(
                        "AllReduce", ALU.add, replica_groups=groups,
                        ins=[cc2_in[n]], outs=[cc2_out[n]])

                emit_final(NSL - 1)

    nc.compile()
    return nc


def kernel(**inputs):
    x = np.asarray(inputs["x"], dtype=np.float32)
    q_w = np.asarray(inputs["q_w"], dtype=np.float32)
    kvd_w = np.asarray(inputs["kvd_w"], dtype=np.float32)
    kvu_w = np.asarray(inputs["kvu_w"], dtype=np.float32)
    proj_w = np.asarray(inputs["proj_w"], dtype=np.float32)
    f1_w = np.asarray(inputs["f1_w"], dtype=np.float32)
    f2_w = np.asarray(inputs["f2_w"], dtype=np.float32)
    ln1_w = np.asarray(inputs["ln1_w"], dtype=np.float32)
    ln1_b = np.asarray(inputs["ln1_b"], dtype=np.float32)
    ln2_w = np.asarray(inputs["ln2_w"], dtype=np.float32)
    ln2_b = np.asarray(inputs["ln2_b"], dtype=np.float32)
    kvln_w = np.asarray(inputs["kvln_w"], dtype=np.float32)
    kvln_b = np.asarray(inputs["kvln_b"], dtype=np.float32)
    proj_b = np.asarray(inputs["proj_b"], dtype=np.float32)
    f1_b = np.asarray(inputs["f1_b"], dtype=np.float32)
    f2_b = np.asarray(inputs["f2_b"], dtype=np.float32)

    flags = (bool(np.allclose(ln1_w, 1) and np.allclose(ln1_b, 0)),
             bool(np.allclose(kvln_w, 1) and np.allclose(kvln_b, 0)),
             bool(np.allclose(ln2_w, 1) and np.allclose(ln2_b, 0)),
             bool(np.allclose(proj_b, 0)),
             bool(np.allclose(f1_b, 0)),
             bool(np.allclose(f2_b, 0)))
    if flags not in _CACHE:
        _CACHE[flags] = _build(flags)
    nc = _CACHE[flags]

    cos, sinf = _rope_tables()
    kvu_v4 = kvu_w.reshape(R, 2, H, D)
    in_maps = []
    for c in range(8):
        b, half = c // 2, c % 2
        hsl = slice(half * HL, (half + 1) * HL)
        m = {
            "x_loc": np.ascontiguousarray(x[b]),
            "qw_loc": np.ascontiguousarray(
                q_w[:, half * HD:(half + 1) * HD]).astype(BF16),
            "kvd_w": kvd_w.astype(BF16),
            "kvu_k": np.ascontiguousarray(
                kvu_v4[:, 0, hsl, :].reshape(R, HD)).astype(BF16),
            "kvu_v": np.ascontiguousarray(
                kvu_v4[:, 1, hsl, :].reshape(R, HD)).astype(BF16),
            "proj_w_loc": np.ascontiguousarray(
                proj_w[half * HD:(half + 1) * HD, :]).astype(BF16),
            "f1_w_loc": np.ascontiguousarray(
                f1_w[:, half * FH:(half + 1) * FH]).astype(BF16),
            "f2_w_loc": np.ascontiguousarray(
                f2_w[half * FH:(half + 1) * FH, :]).astype(BF16),
            "cos_t": cos,
            "sinf_t": sinf,
        }
        if not flags[0]:
            m["ln1_w"], m["ln1_b"] = ln1_w, ln1_b
        if not flags[1]:
            m["kvln_w"], m["kvln_b"] = kvln_w, kvln_b
        if not flags[2]:
            m["ln2_w"], m["ln2_b"] = ln2_w, ln2_b
        if not flags[3]:
            m["proj_b"] = proj_b
        if not flags[4]:
            m["f1_b_loc"] = np.ascontiguousarray(f1_b[half * FH:(half + 1) * FH])
        if not flags[5]:
            m["f2_b"] = f2_b
        in_maps.append(m)

    res = run_bass_kernel_spmd(nc, in_maps, list(range(8)), trace=TRACE)
    kernel.last_result = res
    out = np.stack([res.results[2 * b]["out_loc"] for b in range(B)])
    return out
